# revision 1
# baseline (speedup 1.0000x reference)
"""Cross-attention Trainium2 kernel (8 NeuronCores).

Sharding: core i handles batch b = i//2 and head-group g = i%2 (8 of 16
heads). Each core computes its partial output projection; the host sums the
two head-group partials per batch and adds the bias (unsharding).

Device pipeline per core (all matmuls bf16, fp32 accumulation):
  1. cast x,c to bf16 (SWDGE converting DMA), roundtrip through DRAM,
     DMA-transpose -> xT,cT (features on partitions)
  2. Q = x@Wq, K = c@Wk (natural layout), RoPE applied on-chip,
     roundtrip -> QT,KT (d on partitions); V = c@Wv natural (k on partitions)
     stored with an appended all-ones block column so the AV matmul also
     produces replicated softmax denominators
  3. scores^T = KT.T@QT per head (two heads packed in the 128-partition dim
     via row tiling), exp on ScalarE (scale=1/8 fused, bf16 out), AV
     accumulation -> out^T rows 0:64 + denominator rows 64:128;
     reciprocal + multiply normalizes straight into A^T
  4. Y_partial = A@Wproj_g
"""
import sys

sys.path.insert(0, "/opt/trn_rl_repo")

import numpy as np

import concourse.bass as bass
import concourse.mybir as mybir
from concourse import bacc
from concourse.tile import TileContext
from concourse.bass_utils import run_bass_kernel_spmd

F32 = mybir.dt.float32
BF16 = mybir.dt.bfloat16
AF = mybir.ActivationFunctionType
ALU = mybir.AluOpType

P = 128
DIM = 1024
H = 8          # heads per core
HD = 64        # head dim
QC = 512       # q/k/v columns per core (H*HD)
NX = 1024      # query tokens
NC = 2048      # context tokens
XB = NX // P   # 8 x token blocks
CB = NC // P   # 16 c token blocks
KC = DIM // P  # 8 contraction chunks for projections


def _emit_rope(nc, pool, psum, trig_cos, trig_sin, tb, out_bf):
    """RoPE on a [128, QC] psum tile viewed as [128, H, HD]; writes bf16."""
    pq = psum.rearrange("p (h d) -> p h d", h=H)
    cosb = trig_cos[:, tb, None, :].to_broadcast((P, H, HD))
    sin_lo = trig_sin[:, tb, None, 0:32].to_broadcast((P, H, 32))
    sin_hi = trig_sin[:, tb, None, 32:64].to_broadcast((P, H, 32))
    tmp = pool.tile([P, H, HD], F32, name="rope_tmp", tag="rope_tmp")
    nc.vector.tensor_tensor(tmp[:, :, 0:32], pq[:, :, 32:64], sin_lo, ALU.mult)
    nc.vector.tensor_tensor(tmp[:, :, 32:64], pq[:, :, 0:32], sin_hi, ALU.mult)
    cq = pool.tile([P, H, HD], F32, name="rope_cq", tag="rope_cq")
    nc.vector.tensor_tensor(cq, pq, cosb, ALU.mult)
    ov = out_bf.rearrange("p (h d) -> p h d", h=H)
    nc.vector.tensor_tensor(ov, cq, tmp, ALU.add)


def build_kernel():
    nc = bacc.Bacc("TRN2", target_bir_lowering=False, debug=False)

    x_d = nc.dram_tensor("x", [NX, DIM], F32, kind="ExternalInput")
    c_d = nc.dram_tensor("c", [NC, DIM], F32, kind="ExternalInput")
    wq_d = nc.dram_tensor("wq", [DIM, QC], F32, kind="ExternalInput")
    wk_d = nc.dram_tensor("wk", [DIM, QC], F32, kind="ExternalInput")
    wv_d = nc.dram_tensor("wv", [DIM, QC], F32, kind="ExternalInput")
    wp_d = nc.dram_tensor("wp", [QC, DIM], F32, kind="ExternalInput")
    xpos_d = nc.dram_tensor("xpos", [NX, HD], F32, kind="ExternalInput")
    cpos_d = nc.dram_tensor("cpos", [NC, HD], F32, kind="ExternalInput")
    y_d = nc.dram_tensor("y", [NX, DIM], F32, kind="ExternalOutput")

    with TileContext(nc) as tc:
        with tc.tile_pool(name="persist", bufs=1) as pers, \
             tc.tile_pool(name="stage", bufs=2) as stage, \
             tc.tile_pool(name="dram", bufs=1, space="DRAM") as dram:

            # ---------------- trig tables (natural layout) ----------------
            xpos_sb = pers.tile([P, XB, HD], F32)
            cpos_sb = pers.tile([P, CB, HD], F32)
            nc.sync.dma_start(xpos_sb, xpos_d.rearrange("(o p) d -> p o d", p=P))
            nc.sync.dma_start(cpos_sb, cpos_d.rearrange("(o p) d -> p o d", p=P))
            cosx = pers.tile([P, XB, HD], F32)
            sinx = pers.tile([P, XB, HD], F32)
            cosc = pers.tile([P, CB, HD], F32)
            sinc = pers.tile([P, CB, HD], F32)
            # ACT Sin domain is ~[-pi, pi]: wrap args into range first.
            # cos(t) = sin(t + pi/2)
            PI, TWO_PI = float(np.pi), float(2 * np.pi)
            for pos_sb, sin_t, cos_t in ((xpos_sb, sinx, cosx), (cpos_sb, sinc, cosc)):
                rr = stage.tile(list(pos_sb.shape), F32, name="rr", tag="rr")
                nc.vector.add_range_wrap(rr, pos_sb, 0.0, PI, TWO_PI)
                nc.scalar.activation(sin_t, rr, AF.Sin, scale=1.0)
                rr2 = stage.tile(list(pos_sb.shape), F32, name="rr2", tag="rr")
                nc.vector.add_range_wrap(rr2, pos_sb, PI / 2, PI, TWO_PI)
                nc.scalar.activation(cos_t, rr2, AF.Sin, scale=1.0)
                # signed sin: -sin for d<32 (rotation term sign), +sin for d>=32
                nc.vector.tensor_scalar_mul(sin_t[:, :, 0:32], sin_t[:, :, 0:32], -1.0)

            # ---------------- x -> bf16 dram (converting DMA) -------------
            xbf_dram = dram.tile([NX, DIM], BF16)
            cbf_dram = dram.tile([NC, DIM], BF16)
            for tb in range(XB):
                xbf = stage.tile([P, DIM], BF16, name="xbf", tag="xbf")
                nc.gpsimd.dma_start(xbf, x_d[tb * P:(tb + 1) * P, :])
                nc.sync.dma_start(xbf_dram[tb * P:(tb + 1) * P, :], xbf)

            qbf_dram = dram.tile([NX, QC], BF16)
            kbf_dram = dram.tile([NC, QC], BF16)
            v_aug = pers.tile([P, CB, H, P], BF16)  # [:, :, :, 0:64]=V, 64:128=ones
            nc.vector.memset(v_aug[:, :, :, HD:P], 1.0)
            qT = pers.tile([P, QC // P, NX], BF16)
            kT = pers.tile([P, QC // P, NC], BF16)

            with tc.tile_pool(name="phase_q", bufs=1) as ph_q, \
                 tc.tile_pool(name="ps_proj", bufs=2, space="PSUM") as ps_proj, \
                 tc.tile_pool(name="ps_s", bufs=1, space="PSUM") as ps_s, \
                 tc.tile_pool(name="ps_av", bufs=1, space="PSUM") as ps_av:
                # ------------- Q projection (+RoPE) -----------------------
                wq_bf = ph_q.tile([P, KC, QC], BF16)
                nc.gpsimd.dma_start(wq_bf, wq_d.rearrange("(o p) n -> p o n", p=P))
                xT = ph_q.tile([P, KC, NX], BF16)
                # half-row transposes so Q proj can start after 4 x-blocks
                for half in range(2):
                    rs = slice(half * 512, (half + 1) * 512)
                    for ch in range(KC):
                        nc.sync.dma_start_transpose(xT[:, ch, rs],
                                                    xbf_dram[rs, ch * P:(ch + 1) * P])
                for tb in range(XB):
                    pq = ps_proj.tile([P, QC], F32, name="pq", tag="pp")
                    for kc in range(KC):
                        nc.tensor.matmul(pq, xT[:, kc, tb * P:(tb + 1) * P],
                                         wq_bf[:, kc, :],
                                         start=(kc == 0), stop=(kc == KC - 1))
                    q_bf = stage.tile([P, QC], BF16, name="q_bf", tag="q_bf")
                    _emit_rope(nc, stage, pq, cosx, sinx, tb, q_bf)
                    nc.sync.dma_start(qbf_dram[tb * P:(tb + 1) * P, :], q_bf)
                # qT per (pair, half)
                for p in range(QC // P):
                    for qb in range(2):
                        rs = slice(qb * 512, (qb + 1) * 512)
                        nc.sync.dma_start_transpose(qT[:, p, rs],
                                                    qbf_dram[rs, p * P:(p + 1) * P])

                # c -> bf16 (after the x/q chain is queued)
                for tb in range(CB):
                    cbf = stage.tile([P, DIM], BF16, name="cbf", tag="xbf")
                    nc.gpsimd.dma_start(cbf, c_d[tb * P:(tb + 1) * P, :])
                    nc.sync.dma_start(cbf_dram[tb * P:(tb + 1) * P, :], cbf)
                # output-projection weights (used last; queue after c)
                wp_bf = pers.tile([P, QC // P, DIM], BF16)
                nc.gpsimd.dma_start(wp_bf, wp_d.rearrange("(o p) n -> p o n", p=P))

                # ------------- K/V projections, interleaved per block -----
                with tc.tile_pool(name="phase_kv", bufs=1) as ph_kv:
                    wk_bf = ph_kv.tile([P, KC, QC], BF16)
                    wv_bf = ph_kv.tile([P, KC, QC], BF16)
                    nc.gpsimd.dma_start(wk_bf, wk_d.rearrange("(o p) n -> p o n", p=P))
                    nc.gpsimd.dma_start(wv_bf, wv_d.rearrange("(o p) n -> p o n", p=P))
                    cT = ph_kv.tile([P, KC, NC], BF16)
                    for half in range(2):
                        rs = slice(half * NX, (half + 1) * NX)
                        for ch in range(KC):
                            nc.sync.dma_start_transpose(cT[:, ch, rs],
                                                        cbf_dram[rs, ch * P:(ch + 1) * P])
                    for tb in range(CB):
                        pk = ps_proj.tile([P, QC], F32, name="pk", tag="pp")
                        for kc in range(KC):
                            nc.tensor.matmul(pk, cT[:, kc, tb * P:(tb + 1) * P],
                                             wk_bf[:, kc, :],
                                             start=(kc == 0), stop=(kc == KC - 1))
                        k_bf = stage.tile([P, QC], BF16, name="k_bf", tag="q_bf")
                        _emit_rope(nc, stage, pk, cosc, sinc, tb, k_bf)
                        nc.sync.dma_start(kbf_dram[tb * P:(tb + 1) * P, :], k_bf)
                        pv = ps_proj.tile([P, QC], F32, name="pv", tag="pp")
                        for kc in range(KC):
                            nc.tensor.matmul(pv, cT[:, kc, tb * P:(tb + 1) * P],
                                             wv_bf[:, kc, :],
                                             start=(kc == 0), stop=(kc == KC - 1))
                        nc.vector.tensor_copy(
                            v_aug[:, tb, :, 0:HD],
                            pv.rearrange("p (h d) -> p h d", h=H))
                        # kT quarter transposes as soon as 4 K blocks land
                        if tb % 4 == 3:
                            rs = slice((tb - 3) * P, (tb + 1) * P)
                            for p in range(QC // P):
                                nc.sync.dma_start_transpose(
                                    kT[:, p, rs], kbf_dram[rs, p * P:(p + 1) * P])

                # ---------------- attention -------------------------------
                a_T = pers.tile([P, QC // P, NX], BF16)   # normalized out^T
                for p in range(QC // P):  # head pairs
                    for par in range(2):  # head parity within the pair
                        h = 2 * p + par
                        rows = slice(par * 64, (par + 1) * 64)
                        pav = ps_av.tile([P, NX], F32, name=f"pav_{h}", tag="av")
                        for mm in range(CB // 2):  # two k-blocks per psum tile
                            s2 = ps_s.tile([P, 2, NX], F32, name=f"s2_{h}_{mm}", tag="s")
                            for mi in range(2):
                                m = mm * 2 + mi
                                for qb in range(2):
                                    sl = slice(qb * 512, (qb + 1) * 512)
                                    nc.tensor.matmul(s2[:, mi, sl],
                                                     kT[rows, p, m * P:(m + 1) * P],
                                                     qT[rows, p, sl],
                                                     start=True, stop=True)
                            e2 = stage.tile([P, 2, NX], BF16, name=f"e_{h}_{mm}", tag="e")
                            nc.scalar.activation(e2, s2, AF.Exp, scale=0.125)
                            for mi in range(2):
                                m = mm * 2 + mi
                                for qb in range(2):
                                    sl = slice(qb * 512, (qb + 1) * 512)
                                    nc.tensor.matmul(pav[:, sl], v_aug[:, m, h, :],
                                                     e2[:, mi, sl],
                                                     start=(m == 0), stop=(m == CB - 1))
                        # rows 64:128 of pav hold the replicated denominator
                        recp = stage.tile([P, NX], F32, name=f"rec_{h}", tag="rec", bufs=1)
                        nc.vector.reciprocal(recp[64:128, :], pav[64:128, :])
                        nc.vector.tensor_tensor(a_T[rows, p, :], pav[0:64, :],
                                                recp[64:128, :], ALU.mult)

                # ---------------- output projection -----------------------
                for tb in range(XB):
                    for ob in range(2):
                        py = ps_proj.tile([P, 512], F32, name=f"py_{tb}_{ob}", tag="pp")
                        for kc in range(QC // P):
                            nc.tensor.matmul(py, a_T[:, kc, tb * P:(tb + 1) * P],
                                             wp_bf[:, kc, ob * 512:(ob + 1) * 512],
                                             start=(kc == 0), stop=(kc == QC // P - 1))
                        y_sb = stage.tile([P, 512], F32, name=f"y_{tb}_{ob}", tag="y")
                        nc.vector.tensor_copy(y_sb, py)
                        nc.sync.dma_start(y_d[tb * P:(tb + 1) * P, ob * 512:(ob + 1) * 512],
                                          y_sb)
    nc.compile()
    return nc


_NC_CACHE = None


def make_in_maps(inputs):
    x, c = inputs["x"], inputs["c"]
    Wq, Wkv, Wproj = inputs["Wq"], inputs["Wkv"], inputs["Wproj"]
    in_maps = []
    for core in range(8):
        b, g = core // 2, core % 2
        sl = slice(g * QC, (g + 1) * QC)
        in_maps.append(dict(
            x=np.ascontiguousarray(x[b], np.float32),
            c=np.ascontiguousarray(c[b], np.float32),
            wq=np.ascontiguousarray(Wq[:, sl], np.float32),
            wk=np.ascontiguousarray(Wkv[:, sl], np.float32),
            wv=np.ascontiguousarray(Wkv[:, DIM + g * QC: DIM + (g + 1) * QC], np.float32),
            wp=np.ascontiguousarray(Wproj[sl, :], np.float32),
            xpos=np.ascontiguousarray(inputs["x_pos_embed"], np.float32),
            cpos=np.ascontiguousarray(inputs["c_pos_embed"], np.float32),
        ))
    return in_maps


def kernel(x, c, x_pos_embed, c_pos_embed, Wq, Wkv, Wproj, bproj):
    global _NC_CACHE
    if _NC_CACHE is None:
        _NC_CACHE = build_kernel()
    nc = _NC_CACHE

    B = x.shape[0]
    in_maps = make_in_maps(dict(x=x, c=c, Wq=Wq, Wkv=Wkv, Wproj=Wproj,
                                x_pos_embed=x_pos_embed, c_pos_embed=c_pos_embed))

    res = run_bass_kernel_spmd(nc, in_maps, core_ids=list(range(8)))
    out = np.empty((B, NX, DIM), np.float32)
    bias = np.asarray(bproj, np.float32)
    for b in range(B):
        out[b] = res.results[2 * b]["y"] + res.results[2 * b + 1]["y"] + bias
    return out



# revision 3
# speedup vs baseline: 1.7251x; 1.7251x over previous
"""Cross-attention Trainium2 kernel (8 NeuronCores), v2.

Sharding: core i handles batch b = i//2 and head-group g = i%2 (8 of 16
heads). The host pre-transposes x,c (feature-major), pre-casts all inputs
to bf16, precomputes signed sin/cos RoPE tables, and sums the two
head-group partials per batch (+bias) on the way out.

Device pipeline per core (bf16 matmuls, fp32 accumulation):
  1. load xT,cT,W (no device-side conversion/transpose of inputs)
  2. Q = x@Wq (natural layout), RoPE on DVE, SBUF->SBUF DMA-transpose
     -> qT (head dims on partitions); same for K -> kT; V natural into
     v_aug with an appended all-ones block column so the AV matmul also
     produces replicated softmax denominators
  3. attention per head over k-blocks: scores^T = kT.T@qT (PSUM
     double-buffered), exp on ScalarE (scale=1/8 fused, bf16 out), AV
     accumulation; K/V-proj blocks interleave between attention units
     so the PE keeps streaming while ScalarE works through the exps
  4. reciprocal of the ones-rows normalizes into A^T; Y_partial = A@Wproj_g
"""
import sys

sys.path.insert(0, "/opt/trn_rl_repo")

import numpy as np
import ml_dtypes

import concourse.bass as bass
import concourse.mybir as mybir
from concourse import bacc
from concourse.tile import TileContext
from concourse.bass_utils import run_bass_kernel_spmd

F32 = mybir.dt.float32
BF16 = mybir.dt.bfloat16
AF = mybir.ActivationFunctionType
ALU = mybir.AluOpType

P = 128
DIM = 1024
H = 8          # heads per core
HD = 64        # head dim
QC = 512       # q/k/v columns per core (H*HD)
NX = 1024      # query tokens
NC = 2048      # context tokens
XB = NX // P   # 8 x token blocks
CB = NC // P   # 16 c token blocks
KC = DIM // P  # 8 contraction chunks for projections
MT = QC // P   # 4 head-dim tiles (pairs of heads)


def _emit_rope(nc, pool, psum, trig_cos, trig_sin, tb, out_bf):
    """RoPE on a [128, QC] psum tile viewed as [128, H, HD]; writes bf16.

    trig tables are [128, nblk, HD] f32 with the rotation sign folded into
    sin (host-precomputed): out = p*cos + rot(p)*sin_signed."""
    pq = psum.rearrange("p (h d) -> p h d", h=H)
    cosb = trig_cos[:, tb, None, :].to_broadcast((P, H, HD))
    sin_lo = trig_sin[:, tb, None, 0:32].to_broadcast((P, H, 32))
    sin_hi = trig_sin[:, tb, None, 32:64].to_broadcast((P, H, 32))
    tmp = pool.tile([P, H, HD], F32, name="rope_tmp", tag="rope_tmp")
    nc.vector.tensor_tensor(tmp[:, :, 0:32], pq[:, :, 32:64], sin_lo, ALU.mult)
    nc.vector.tensor_tensor(tmp[:, :, 32:64], pq[:, :, 0:32], sin_hi, ALU.mult)
    cq = pool.tile([P, H, HD], F32, name="rope_cq", tag="rope_cq")
    nc.vector.tensor_tensor(cq, pq, cosb, ALU.mult)
    ov = out_bf.rearrange("p (h d) -> p h d", h=H)
    nc.vector.tensor_tensor(ov, cq, tmp, ALU.add)


def build_kernel():
    nc = bacc.Bacc("TRN2", target_bir_lowering=False, debug=False)

    xt_d = nc.dram_tensor("xt", [DIM, NX], BF16, kind="ExternalInput")
    ct_d = nc.dram_tensor("ct", [DIM, NC], BF16, kind="ExternalInput")
    wq_d = nc.dram_tensor("wq", [DIM, QC], BF16, kind="ExternalInput")
    wk_d = nc.dram_tensor("wk", [DIM, QC], BF16, kind="ExternalInput")
    wv_d = nc.dram_tensor("wv", [DIM, QC], BF16, kind="ExternalInput")
    wp_d = nc.dram_tensor("wp", [QC, DIM], BF16, kind="ExternalInput")
    cosx_d = nc.dram_tensor("cosx", [NX, HD], F32, kind="ExternalInput")
    sinx_d = nc.dram_tensor("sinx", [NX, HD], F32, kind="ExternalInput")
    cosc_d = nc.dram_tensor("cosc", [NC, HD], F32, kind="ExternalInput")
    sinc_d = nc.dram_tensor("sinc", [NC, HD], F32, kind="ExternalInput")
    y_d = nc.dram_tensor("y", [NX, DIM], F32, kind="ExternalOutput")

    with TileContext(nc) as tc:
        with tc.tile_pool(name="persist", bufs=1) as pers, \
             tc.tile_pool(name="stage", bufs=2) as stage, \
             tc.tile_pool(name="ps_proj", bufs=2, space="PSUM") as ps_proj, \
             tc.tile_pool(name="ps_s", bufs=2, space="PSUM") as ps_s, \
             tc.tile_pool(name="ps_av", bufs=1, space="PSUM") as ps_av:

            # ------------- persistent SBUF tensors -------------
            xT = pers.tile([P, KC, NX], BF16)
            cT = pers.tile([P, KC, NC], BF16)
            wq_sb = pers.tile([P, KC, QC], BF16)
            wk_sb = pers.tile([P, KC, QC], BF16)
            wv_sb = pers.tile([P, KC, QC], BF16)
            wp_sb = pers.tile([P, MT, DIM], BF16)
            cosx_sb = pers.tile([P, XB, HD], F32)
            sinx_sb = pers.tile([P, XB, HD], F32)
            cosc_sb = pers.tile([P, CB, HD], F32)
            sinc_sb = pers.tile([P, CB, HD], F32)
            qT = pers.tile([P, MT, NX], BF16)
            kT = pers.tile([P, MT, NC], BF16)
            v_aug = pers.tile([P, CB, H, P], BF16)  # [...,0:64]=V, 64:128=ones
            a_T = pers.tile([P, MT, NX], BF16)

            # ------------- input DMAs, spread over 4 queues -------------
            # sync(SP): x-side; scalar(Act): cT; vector(DVE): K/V/P weights;
            # gpsimd(Pool): trig tables.  Issue in dependency order.
            nc.sync.dma_start(wq_sb, wq_d.rearrange("(o p) n -> p o n", p=P))
            for half in range(2):
                ks = slice(half * 4, half * 4 + 4)
                nc.sync.dma_start(xT[:, ks, :],
                                  xt_d.rearrange("(o p) n -> p o n", p=P)[:, ks, :])
            nc.gpsimd.dma_start(cosx_sb, cosx_d.rearrange("(o p) d -> p o d", p=P))
            nc.gpsimd.dma_start(sinx_sb, sinx_d.rearrange("(o p) d -> p o d", p=P))
            for q in range(4):
                ks = slice(q * 2, q * 2 + 2)
                nc.scalar.dma_start(cT[:, ks, :],
                                    ct_d.rearrange("(o p) n -> p o n", p=P)[:, ks, :])
            nc.gpsimd.dma_start(wk_sb, wk_d.rearrange("(o p) n -> p o n", p=P))
            nc.gpsimd.dma_start(wv_sb, wv_d.rearrange("(o p) n -> p o n", p=P))
            nc.gpsimd.dma_start(cosc_sb, cosc_d.rearrange("(o p) d -> p o d", p=P))
            nc.gpsimd.dma_start(sinc_sb, sinc_d.rearrange("(o p) d -> p o d", p=P))
            nc.gpsimd.dma_start(wp_sb, wp_d.rearrange("(o p) n -> p o n", p=P))

            # ones block for the AV denominator trick (Pool engine is idle)
            nc.gpsimd.memset(v_aug[:, :, :, HD:P], 1.0)

            # ------------- Q projection + RoPE + transpose -------------
            for tb in range(XB):
                pq = ps_proj.tile([P, QC], F32, name=f"pq{tb}", tag="pp")
                for kc in range(KC):
                    nc.tensor.matmul(pq, xT[:, kc, tb * P:(tb + 1) * P],
                                     wq_sb[:, kc, :],
                                     start=(kc == 0), stop=(kc == KC - 1))
                q_bf = stage.tile([P, QC], BF16, name=f"qbf{tb}", tag="q_bf",
                                  bufs=3)
                _emit_rope(nc, stage, pq, cosx_sb, sinx_sb, tb, q_bf)
                nc.sync.dma_start_transpose(qT[:, 0:MT, tb * P:(tb + 1) * P], q_bf)

            # ------------- K/V projection blocks -------------
            def kv_block(tb):
                pk = ps_proj.tile([P, QC], F32, name=f"pk{tb}", tag="pp")
                for kc in range(KC):
                    nc.tensor.matmul(pk, cT[:, kc, tb * P:(tb + 1) * P],
                                     wk_sb[:, kc, :],
                                     start=(kc == 0), stop=(kc == KC - 1))
                k_bf = stage.tile([P, QC], BF16, name=f"kbf{tb}", tag="q_bf",
                                  bufs=3)
                _emit_rope(nc, stage, pk, cosc_sb, sinc_sb, tb, k_bf)
                nc.sync.dma_start_transpose(kT[:, 0:MT, tb * P:(tb + 1) * P], k_bf)
                pv = ps_proj.tile([P, QC], F32, name=f"pv{tb}", tag="pp")
                for kc in range(KC):
                    nc.tensor.matmul(pv, cT[:, kc, tb * P:(tb + 1) * P],
                                     wv_sb[:, kc, :],
                                     start=(kc == 0), stop=(kc == KC - 1))
                nc.vector.tensor_copy(v_aug[:, tb, :, 0:HD],
                                      pv.rearrange("p (h d) -> p h d", h=H))

            # ------------- attention unit (head h, k-block m) -------------
            pavs = {}

            def att_unit(h, m):
                p, par = h // 2, h % 2
                rows = slice(par * HD, (par + 1) * HD)
                if m == 0:
                    pavs[h] = ps_av.tile([P, NX], F32, name=f"pav{h}", tag="av")
                pav = pavs[h]
                s = ps_s.tile([P, NX], F32, name=f"s{h}_{m}", tag="s")
                for qb in range(2):
                    sl = slice(qb * 512, (qb + 1) * 512)
                    nc.tensor.matmul(s[:, sl], kT[rows, p, m * P:(m + 1) * P],
                                     qT[rows, p, sl], start=True, stop=True)
                e2 = stage.tile([P, NX], BF16, name=f"e{h}_{m}", tag="e", bufs=3)
                nc.scalar.activation(e2, s, AF.Exp, scale=0.125)
                for qb in range(2):
                    sl = slice(qb * 512, (qb + 1) * 512)
                    nc.tensor.matmul(pav[:, sl], v_aug[:, m, h, :], e2[:, sl],
                                     start=(m == 0), stop=(m == CB - 1))
                if m == CB - 1:
                    recp = stage.tile([P, NX], F32, name=f"rec{h}", tag="rec",
                                      bufs=1)
                    nc.vector.reciprocal(recp[64:128, :], pav[64:128, :])
                    nc.vector.tensor_tensor(a_T[rows, p, :], pav[0:64, :],
                                            recp[64:128, :], ALU.mult)

            # ------------- interleaved K/V + attention schedule -------------
            units = [(h, m) for h in range(H) for m in range(CB)]
            ui = 0
            for tb in range(4):
                kv_block(tb)
            for tb in range(4, CB):
                kv_block(tb)
                emitted = 0
                while ui < len(units) and emitted < 3 and units[ui][1] <= tb - 2:
                    att_unit(*units[ui])
                    ui += 1
                    emitted += 1
            while ui < len(units):
                att_unit(*units[ui])
                ui += 1

            # ------------- output projection -------------
            for tb in range(XB):
                for ob in range(2):
                    py = ps_proj.tile([P, 512], F32, name=f"py{tb}_{ob}",
                                      tag="pp")
                    for kc in range(MT):
                        nc.tensor.matmul(py, a_T[:, kc, tb * P:(tb + 1) * P],
                                         wp_sb[:, kc, ob * 512:(ob + 1) * 512],
                                         start=(kc == 0), stop=(kc == MT - 1))
                    y_sb = stage.tile([P, 512], F32, name=f"y{tb}_{ob}", tag="y")
                    nc.scalar.copy(y_sb, py)
                    nc.sync.dma_start(y_d[tb * P:(tb + 1) * P,
                                          ob * 512:(ob + 1) * 512], y_sb)
    nc.compile()
    return nc


_NC_CACHE = None
BF = ml_dtypes.bfloat16


def make_in_maps(inputs):
    x, c = inputs["x"], inputs["c"]
    Wq, Wkv, Wproj = inputs["Wq"], inputs["Wkv"], inputs["Wproj"]

    def bft(a):  # bf16 transpose, contiguous
        return np.ascontiguousarray(np.asarray(a, np.float32).T.astype(BF))

    def bf(a):
        return np.ascontiguousarray(np.asarray(a, np.float32).astype(BF))

    thx = np.asarray(inputs["x_pos_embed"], np.float32)
    thc = np.asarray(inputs["c_pos_embed"], np.float32)
    cosx, sinx = np.cos(thx), np.sin(thx)
    cosc, sinc = np.cos(thc), np.sin(thc)
    sinx = sinx.copy()
    sinc = sinc.copy()
    sinx[:, 0:HD // 2] *= -1.0   # rotation sign folded into the table
    sinc[:, 0:HD // 2] *= -1.0

    xt = [bft(x[b]) for b in range(4)]
    ct = [bft(c[b]) for b in range(4)]
    in_maps = []
    for core in range(8):
        b, g = core // 2, core % 2
        sl = slice(g * QC, (g + 1) * QC)
        in_maps.append(dict(
            xt=xt[b],
            ct=ct[b],
            wq=bf(Wq[:, sl]),
            wk=bf(Wkv[:, sl]),
            wv=bf(Wkv[:, DIM + g * QC: DIM + (g + 1) * QC]),
            wp=bf(Wproj[sl, :]),
            cosx=np.ascontiguousarray(cosx),
            sinx=np.ascontiguousarray(sinx),
            cosc=np.ascontiguousarray(cosc),
            sinc=np.ascontiguousarray(sinc),
        ))
    return in_maps


def kernel(x, c, x_pos_embed, c_pos_embed, Wq, Wkv, Wproj, bproj):
    global _NC_CACHE
    if _NC_CACHE is None:
        _NC_CACHE = build_kernel()
    nc = _NC_CACHE

    B = x.shape[0]
    in_maps = make_in_maps(dict(x=x, c=c, Wq=Wq, Wkv=Wkv, Wproj=Wproj,
                                x_pos_embed=x_pos_embed,
                                c_pos_embed=c_pos_embed))

    res = run_bass_kernel_spmd(nc, in_maps, core_ids=list(range(8)))
    out = np.empty((B, NX, DIM), np.float32)
    bias = np.asarray(bproj, np.float32)
    for b in range(B):
        out[b] = res.results[2 * b]["y"] + res.results[2 * b + 1]["y"] + bias
    return out


# revision 4
# speedup vs baseline: 1.7452x; 1.0117x over previous
"""Cross-attention Trainium2 kernel (8 NeuronCores), v3.

Sharding: core i handles batch b = i//2 and head-group g = i%2 (8 of 16
heads). The host pre-transposes x,c (feature-major), pre-casts all inputs
to bf16, precomputes signed sin/cos RoPE tables, and sums the two
head-group partials per batch (+bias) on the way out.

Device pipeline per core (bf16 matmuls, fp32 accumulation):
  1. inputs DMA'd in consumption order (the DMA engines are a serial
     resource: chunk order == schedule)
  2. Q = x@Wq (natural layout), RoPE on DVE, SBUF->SBUF DMA-transpose
     -> qT (head dims on partitions); same for K -> kT; V natural into
     v_aug with an appended all-ones block column so the AV matmul also
     produces replicated softmax denominators
  3. attention per head over k-blocks: scores^T = kT.T@qT (PSUM
     double-buffered), exp on ScalarE (scale=1/8 fused, bf16 out), AV
     accumulation; K/V-proj blocks and the first half of the output
     projection interleave between attention units as PE filler while
     ScalarE works through the exps
  4. reciprocal of the ones-rows normalizes into A^T; Y = A@Wproj_g in
     two half-contraction passes (partial kept in bf16 SBUF)
"""
import sys

sys.path.insert(0, "/opt/trn_rl_repo")

import numpy as np
import ml_dtypes

import concourse.bass as bass
import concourse.mybir as mybir
from concourse import bacc
from concourse.tile import TileContext
from concourse.bass_utils import run_bass_kernel_spmd

F32 = mybir.dt.float32
BF16 = mybir.dt.bfloat16
AF = mybir.ActivationFunctionType
ALU = mybir.AluOpType

P = 128
DIM = 1024
H = 8          # heads per core
HD = 64        # head dim
QC = 512       # q/k/v columns per core (H*HD)
NX = 1024      # query tokens
NC = 2048      # context tokens
XB = NX // P   # 8 x token blocks
CB = NC // P   # 16 c token blocks
KC = DIM // P  # 8 contraction chunks for projections
MT = QC // P   # 4 head-dim tiles (pairs of heads)


def _emit_rope(nc, pool, psum, trig_cos, trig_sin, tb, out_bf):
    """RoPE on a [128, QC] psum tile viewed as [128, H, HD]; writes bf16.

    trig tables are [128, nblk, HD] bf16 with the rotation sign folded
    into sin (host-precomputed): out = p*cos + rot(p)*sin_signed."""
    pq = psum.rearrange("p (h d) -> p h d", h=H)
    cosb = trig_cos[:, tb, None, :].to_broadcast((P, H, HD))
    sin_lo = trig_sin[:, tb, None, 0:32].to_broadcast((P, H, 32))
    sin_hi = trig_sin[:, tb, None, 32:64].to_broadcast((P, H, 32))
    tmp = pool.tile([P, H, HD], F32, name="rope_tmp", tag="rope_tmp")
    nc.vector.tensor_tensor(tmp[:, :, 0:32], pq[:, :, 32:64], sin_lo, ALU.mult)
    nc.vector.tensor_tensor(tmp[:, :, 32:64], pq[:, :, 0:32], sin_hi, ALU.mult)
    cq = pool.tile([P, H, HD], F32, name="rope_cq", tag="rope_cq")
    nc.vector.tensor_tensor(cq, pq, cosb, ALU.mult)
    ov = out_bf.rearrange("p (h d) -> p h d", h=H)
    nc.vector.tensor_tensor(ov, cq, tmp, ALU.add)


def build_kernel():
    nc = bacc.Bacc("TRN2", target_bir_lowering=False, debug=False)

    xt_d = nc.dram_tensor("xt", [DIM, NX], BF16, kind="ExternalInput")
    ct_d = nc.dram_tensor("ct", [DIM, NC], BF16, kind="ExternalInput")
    wq_d = nc.dram_tensor("wq", [DIM, QC], BF16, kind="ExternalInput")
    wk_d = nc.dram_tensor("wk", [DIM, QC], BF16, kind="ExternalInput")
    wv_d = nc.dram_tensor("wv", [DIM, QC], BF16, kind="ExternalInput")
    wp_d = nc.dram_tensor("wp", [QC, DIM], BF16, kind="ExternalInput")
    cosx_d = nc.dram_tensor("cosx", [NX, HD], BF16, kind="ExternalInput")
    sinx_d = nc.dram_tensor("sinx", [NX, HD], BF16, kind="ExternalInput")
    cosc_d = nc.dram_tensor("cosc", [NC, HD], BF16, kind="ExternalInput")
    sinc_d = nc.dram_tensor("sinc", [NC, HD], BF16, kind="ExternalInput")
    y_d = nc.dram_tensor("y", [NX, DIM], BF16, kind="ExternalOutput")

    with TileContext(nc) as tc:
        with tc.tile_pool(name="persist", bufs=1) as pers, \
             tc.tile_pool(name="stage", bufs=2) as stage, \
             tc.tile_pool(name="ps_proj", bufs=2, space="PSUM") as ps_proj, \
             tc.tile_pool(name="ps_s", bufs=2, space="PSUM") as ps_s, \
             tc.tile_pool(name="ps_av", bufs=1, space="PSUM") as ps_av:

            # ------------- persistent SBUF tensors -------------
            cT = pers.tile([P, KC, NC], BF16)
            wq_sb = pers.tile([P, KC, QC], BF16)
            wk_sb = pers.tile([P, KC, QC], BF16)
            wv_sb = pers.tile([P, KC, QC], BF16)
            wp_sb = pers.tile([P, MT, DIM], BF16)
            cosx_sb = pers.tile([P, XB, HD], BF16)
            sinx_sb = pers.tile([P, XB, HD], BF16)
            cosc_sb = pers.tile([P, CB, HD], BF16)
            sinc_sb = pers.tile([P, CB, HD], BF16)
            qT = pers.tile([P, MT, NX], BF16)
            kT = pers.tile([P, MT, NC], BF16)
            v_aug = pers.tile([P, CB, H, P], BF16)  # [...,0:64]=V, 64:128=ones
            a_T = pers.tile([P, MT, NX], BF16)

            wq_v = wq_d.rearrange("(o p) n -> p o n", p=P)
            xt_v = xt_d.rearrange("(o p) n -> p o n", p=P)
            ct_v = ct_d.rearrange("(o p) n -> p o n", p=P)

            with tc.tile_pool(name="xpool", bufs=1) as xpool:
                xT = xpool.tile([P, KC, NX], BF16)

                # ---- input DMAs in consumption order ----
                # SP queue: the Q-phase critical path, chunked per-kc.
                for kc in range(KC):
                    nc.sync.dma_start(wq_sb[:, kc, :], wq_v[:, kc, :])
                    nc.sync.dma_start(xT[:, kc, :], xt_v[:, kc, :])
                    if kc == 2:
                        nc.sync.dma_start(
                            cosx_sb, cosx_d.rearrange("(o p) d -> p o d", p=P))
                        nc.sync.dma_start(
                            sinx_sb, sinx_d.rearrange("(o p) d -> p o d", p=P))
                # Pool queue (SWDGE): K/V-side, token-pair chunks of cT.
                nc.gpsimd.dma_start(wk_sb, wk_d.rearrange("(o p) n -> p o n", p=P))
                for ch in range(CB // 2):
                    sl = slice(ch * 2 * P, (ch + 1) * 2 * P)
                    nc.gpsimd.dma_start(cT[:, :, sl], ct_v[:, :, sl])
                    if ch == 0:
                        nc.gpsimd.dma_start(
                            wv_sb, wv_d.rearrange("(o p) n -> p o n", p=P))
                    if ch == 1:
                        nc.gpsimd.dma_start(
                            cosc_sb, cosc_d.rearrange("(o p) d -> p o d", p=P))
                        nc.gpsimd.dma_start(
                            sinc_sb, sinc_d.rearrange("(o p) d -> p o d", p=P))
                nc.gpsimd.dma_start(wp_sb, wp_d.rearrange("(o p) n -> p o n", p=P))

                # ---- Q projection + RoPE + transpose ----
                for tb in range(XB):
                    pq = ps_proj.tile([P, QC], F32, name=f"pq{tb}", tag="pp")
                    for kc in range(KC):
                        nc.tensor.matmul(pq, xT[:, kc, tb * P:(tb + 1) * P],
                                         wq_sb[:, kc, :],
                                         start=(kc == 0), stop=(kc == KC - 1))
                    q_bf = stage.tile([P, QC], BF16, name=f"qbf{tb}",
                                      tag="q_bf", bufs=3)
                    _emit_rope(nc, stage, pq, cosx_sb, sinx_sb, tb, q_bf)
                    nc.sync.dma_start_transpose(
                        qT[:, 0:MT, tb * P:(tb + 1) * P], q_bf)

            with tc.tile_pool(name="ypool", bufs=1) as ypool:
                yA = ypool.tile([P, XB, DIM], BF16)  # partial Y (pairs 0,1)

                # ---- K/V projection block ----
                def kv_block(tb):
                    nc.gpsimd.memset(v_aug[:, tb, :, HD:P], 1.0)
                    pk = ps_proj.tile([P, QC], F32, name=f"pk{tb}", tag="pp")
                    for kc in range(KC):
                        nc.tensor.matmul(pk, cT[:, kc, tb * P:(tb + 1) * P],
                                         wk_sb[:, kc, :],
                                         start=(kc == 0), stop=(kc == KC - 1))
                    k_bf = stage.tile([P, QC], BF16, name=f"kbf{tb}",
                                      tag="q_bf", bufs=3)
                    _emit_rope(nc, stage, pk, cosc_sb, sinc_sb, tb, k_bf)
                    nc.sync.dma_start_transpose(
                        kT[:, 0:MT, tb * P:(tb + 1) * P], k_bf)
                    pv = ps_proj.tile([P, QC], F32, name=f"pv{tb}", tag="pp")
                    for kc in range(KC):
                        nc.tensor.matmul(pv, cT[:, kc, tb * P:(tb + 1) * P],
                                         wv_sb[:, kc, :],
                                         start=(kc == 0), stop=(kc == KC - 1))
                    nc.scalar.copy(v_aug[:, tb, :, 0:HD],
                                   pv.rearrange("p (h d) -> p h d", h=H))

                # ---- attention unit (head h, k-block m) ----
                pavs = {}

                def att_unit(h, m):
                    p, par = h // 2, h % 2
                    rows = slice(par * HD, (par + 1) * HD)
                    if m == 0:
                        pavs[h] = ps_av.tile([P, NX], F32, name=f"pav{h}",
                                             tag="av")
                    pav = pavs[h]
                    s = ps_s.tile([P, NX], F32, name=f"s{h}_{m}", tag="s")
                    for qb in range(2):
                        sl = slice(qb * 512, (qb + 1) * 512)
                        nc.tensor.matmul(s[:, sl],
                                         kT[rows, p, m * P:(m + 1) * P],
                                         qT[rows, p, sl], start=True, stop=True)
                    e2 = stage.tile([P, NX], BF16, name=f"e{h}_{m}", tag="e",
                                    bufs=3)
                    nc.scalar.activation(e2, s, AF.Exp, scale=0.125)
                    for qb in range(2):
                        sl = slice(qb * 512, (qb + 1) * 512)
                        nc.tensor.matmul(pav[:, sl], v_aug[:, m, h, :],
                                         e2[:, sl],
                                         start=(m == 0), stop=(m == CB - 1))
                    if m == CB - 1:
                        recp = stage.tile([P, NX], F32, name=f"rec{h}",
                                          tag="rec", bufs=1)
                        nc.vector.reciprocal(recp[64:128, :], pav[64:128, :])
                        nc.vector.tensor_tensor(a_T[rows, p, :], pav[0:64, :],
                                                recp[64:128, :], ALU.mult)

                # ---- output projection, two half-contraction passes ----
                def proj_unit(i, first):
                    tb, ob = divmod(i, 2)
                    sl = slice(ob * 512, (ob + 1) * 512)
                    tsl = slice(tb * P, (tb + 1) * P)
                    py = ps_proj.tile([P, 512], F32,
                                      name=f"py{tb}_{ob}_{int(first)}",
                                      tag="pp")
                    kcs = (0, 1) if first else (2, 3)
                    for kc in kcs:
                        nc.tensor.matmul(py, a_T[:, kc, tsl], wp_sb[:, kc, sl],
                                         start=(kc == kcs[0]),
                                         stop=(kc == kcs[1]))
                    if first:
                        nc.vector.tensor_copy(yA[:, tb, sl], py)
                    else:
                        y_bf = stage.tile([P, 512], BF16, name=f"y{tb}_{ob}",
                                          tag="y", bufs=3)
                        nc.vector.tensor_tensor(y_bf, py, yA[:, tb, sl],
                                                ALU.add)
                        nc.sync.dma_start(y_d[tsl, sl], y_bf)

                # ---- interleaved schedule ----
                units = [(h, m) for h in range(H) for m in range(CB)]
                ui = 0
                for tb in range(3):
                    kv_block(tb)
                for tb in range(3, CB):
                    kv_block(tb)
                    emitted = 0
                    while (ui < len(units) and emitted < 3
                           and units[ui][1] <= tb - 2):
                        att_unit(*units[ui])
                        ui += 1
                        emitted += 1
                pa = 0
                di = 0
                while ui < len(units):
                    att_unit(*units[ui])
                    h = units[ui][0]
                    ui += 1
                    if h >= 4:
                        di += 1
                        if pa < 16 and di % 4 == 0:
                            proj_unit(pa, first=True)
                            pa += 1
                while pa < 16:
                    proj_unit(pa, first=True)
                    pa += 1
                for i in range(16):
                    proj_unit(i, first=False)
    nc.compile()
    return nc


_NC_CACHE = None
BF = ml_dtypes.bfloat16


def make_in_maps(inputs):
    x, c = inputs["x"], inputs["c"]
    Wq, Wkv, Wproj = inputs["Wq"], inputs["Wkv"], inputs["Wproj"]

    def bft(a):  # bf16 transpose, contiguous
        return np.ascontiguousarray(np.asarray(a, np.float32).T.astype(BF))

    def bf(a):
        return np.ascontiguousarray(np.asarray(a, np.float32).astype(BF))

    thx = np.asarray(inputs["x_pos_embed"], np.float32)
    thc = np.asarray(inputs["c_pos_embed"], np.float32)
    cosx, sinx = np.cos(thx), np.sin(thx).copy()
    cosc, sinc = np.cos(thc), np.sin(thc).copy()
    sinx[:, 0:HD // 2] *= -1.0   # rotation sign folded into the table
    sinc[:, 0:HD // 2] *= -1.0

    xt = [bft(x[b]) for b in range(4)]
    ct = [bft(c[b]) for b in range(4)]
    in_maps = []
    for core in range(8):
        b, g = core // 2, core % 2
        sl = slice(g * QC, (g + 1) * QC)
        in_maps.append(dict(
            xt=xt[b],
            ct=ct[b],
            wq=bf(Wq[:, sl]),
            wk=bf(Wkv[:, sl]),
            wv=bf(Wkv[:, DIM + g * QC: DIM + (g + 1) * QC]),
            wp=bf(Wproj[sl, :]),
            cosx=bf(cosx),
            sinx=bf(sinx),
            cosc=bf(cosc),
            sinc=bf(sinc),
        ))
    return in_maps


def kernel(x, c, x_pos_embed, c_pos_embed, Wq, Wkv, Wproj, bproj):
    global _NC_CACHE
    if _NC_CACHE is None:
        _NC_CACHE = build_kernel()
    nc = _NC_CACHE

    B = x.shape[0]
    in_maps = make_in_maps(dict(x=x, c=c, Wq=Wq, Wkv=Wkv, Wproj=Wproj,
                                x_pos_embed=x_pos_embed,
                                c_pos_embed=c_pos_embed))

    res = run_bass_kernel_spmd(nc, in_maps, core_ids=list(range(8)))
    out = np.empty((B, NX, DIM), np.float32)
    bias = np.asarray(bproj, np.float32)
    for b in range(B):
        out[b] = (np.asarray(res.results[2 * b]["y"], np.float32)
                  + np.asarray(res.results[2 * b + 1]["y"], np.float32)
                  + bias)
    return out
